# revision 65
# baseline (speedup 1.0000x reference)
"""Sharded causal attention (decode-append) kernel for 8 NeuronCores, v3.

Head-parallel: core c owns heads 4c..4c+3. Per-core work: 4 heads x
[512 q] x [4096 ctx] x [128 d] QK/softmax/AV, fp16 operands, fp32 acc.

Design (vs the 118.5us v1 baseline):
- DMA: resident whole-tensor loads with 2-8KB rows (~250-370GB/s vs
  ~90GB/s for v1's 1KB-row chunk loads), one ordered sync queue whose
  first pieces are sized so head 0 can start ~1us after the ~3.4us
  framework preamble.
- Scores in [128,1536] PSUM tiles (3 t-blocks): 11 ScalarE exps per
  head instead of 18 (ACT is 0.83ns/elem + ~0.3us/instr and was the
  v1 co-bottleneck at 85us busy).
- Exp offload: the first 2 score tiles per head (t in [0,768)) are
  computed on the otherwise-overloaded-path-free DVE as an fp16-domain
  Schraudolph exp: int16(x*A16 + B16) bit-viewed as fp16. One
  tensor_scalar per tile, no convert pass (the int16 tile is fed to
  the AV matmul via .bitcast(f16) - verified legal on hw). Adds 8e-3
  rel err (gate 2e-2); C16 calibrated for truncating int conversion
  (hw matches numpy trunc - verified on v2).
- Softmax denominators: fold adds split between GpSimd (tiles 0-4,
  1.2us/add but otherwise idle) and DVE (rest, 0.55us/add), one
  ones-matmul per ~4 tiles into a [1,512] PSUM row.
- AV/folds lag the QK/exp of the next tile by one tile so PE never
  sits on an exp dependency (in-order engine queues).
- No on-chip epilogue: AV accumulators [d,s] are copied PSUM->SBUF on
  ScalarE, denominator rows on DVE, both DMAed raw; the host divides.
- PSUM: 2x sc(3 banks) + av(1) + sum(1) = 8 banks exactly.
"""

import sys

if "/opt/trn_rl_repo" not in sys.path:
    sys.path.insert(0, "/opt/trn_rl_repo")

import numpy as np

NUM_HEADS = 32
HEAD = 128
HIDDEN = NUM_HEADS * HEAD
MAX_SEQ = 4096
N_CORES = 8
HEADS_PER_CORE = NUM_HEADS // N_CORES          # 4
CW = HEADS_PER_CORE * HEAD                     # 512 columns per core
SEQ = 512
OFFSET = 3584
CTX = OFFSET + SEQ                             # 4096
TBLK = 128
NTB = CTX // TBLK                              # 32 t-blocks per head
PREFIX_TB = OFFSET // TBLK                     # 28 unmasked blocks
SCALE = float(1.0 / np.sqrt(np.float32(HEAD)))
MASK_NEG = -1.0e9

# Schraudolph fp16 exp offload: these 3-block tiles per head are exp'd
# on DVE instead of ScalarE. Late tiles chosen so the ScalarE act chain
# starts at tile 0 (it is the per-head critical path).
OFF_TILES = (3, 8)
SCH_C16 = 59.4
SCH_A16 = float(np.log2(np.e) * 1024.0 * SCALE)
SCH_B16 = float(15.0 * 1024.0 - SCH_C16)

TILES = ([[3 * i, 3 * i + 1, 3 * i + 2] for i in range(9)]
         + [[27, 28, 29], [30, 31]])

def _bw(tb):     # block width (query count)
    return SEQ - 128 * (tb - PREFIX_TB) if tb >= PREFIX_TB else SEQ

def _soff(tb):   # query offset
    return 128 * (tb - PREFIX_TB) if tb >= PREFIX_TB else 0

_CACHE: dict = {}


def _build():
    import concourse.bacc as bacc
    import concourse.tile as tile
    from concourse import mybir
    from concourse.vector_clock import ScopedClock

    def _lean_drain_and_barrier(self, tick_clock, wait_clock):
        nc = self.nc
        drain_inst = nc.sync.drain()
        wait_clock.add_sem_waits(
            drain_inst.ins, ScopedClock({None: tick_clock.global_clock}))
        nc.all_engine_barrier()
        popped = nc._tile_sem_poison_stack.pop()
        assert popped is self._sem_poison

        sems = list(self.sems.allocated().values())
        sem_nums = sorted(s.num if hasattr(s, "num") else s for s in sems)
        engines = [nc.gpsimd, nc.vector, nc.scalar, nc.tensor, nc.sync]
        ranges = []
        start = prev = None
        for n in sem_nums:
            if prev is None or n != prev + 1:
                if prev is not None:
                    ranges.append(range(start, prev + 1))
                start = n
            prev = n
        if prev is not None:
            ranges.append(range(start, prev + 1))
        for r in ranges:
            nc.gpsimd.dma_reset(r)
        chunks = []
        for r in ranges:
            vals = list(r)
            k = max(1, len(vals) // len(engines) + 1)
            for i in range(0, len(vals), k):
                seg = vals[i:i + k]
                chunks.append(range(seg[0], seg[-1] + 1))
        for i, r in enumerate(chunks):
            engines[i % len(engines)].sem_clear(r)
        nc._state.prepend_free_semaphores(sem_nums)
        for poison_set in nc._tile_sem_poison_stack:
            poison_set.update(sem_nums)

    tile.TileContext._drain_and_barrier = _lean_drain_and_barrier

    import concourse.bass as _bassmod
    _bassmod.is_customcomms_rdh_enabled = lambda: True

    F32 = mybir.dt.float32
    F16 = mybir.dt.float16
    I16 = mybir.dt.int16
    EXP = mybir.ActivationFunctionType.Exp
    ADD = mybir.AluOpType.add
    MULT = mybir.AluOpType.mult

    nc = bacc.Bacc()
    qt_d = nc.dram_tensor("qt", [128, HEADS_PER_CORE * SEQ], F16,
                          kind="ExternalInput")
    kt_d = nc.dram_tensor("kt", [HEADS_PER_CORE, 128, CTX], F16,
                          kind="ExternalInput")
    vp_d = nc.dram_tensor("vp", [HEADS_PER_CORE // 2, 128, 8 * 1024], F16,
                          kind="ExternalInput")
    outp_d = nc.dram_tensor("outp", [HEADS_PER_CORE, 128, SEQ], F32,
                            kind="ExternalOutput")
    sums_d = nc.dram_tensor("sums", [HEADS_PER_CORE, 1, SEQ], F32,
                            kind="ExternalOutput")

    with tile.TileContext(nc) as tc:
        with (
            tc.tile_pool(name="consts", bufs=1) as consts,
            tc.tile_pool(name="kq", bufs=1) as kq,
            tc.tile_pool(name="vv", bufs=1) as vv,
            tc.tile_pool(name="epool", bufs=8) as epool,
            tc.tile_pool(name="w16p", bufs=3) as w16p,
            tc.tile_pool(name="fpool", bufs=4) as fpool,
            tc.tile_pool(name="upool", bufs=12) as upool,
            tc.tile_pool(name="opool", bufs=2) as opool,
            tc.tile_pool(name="pssc", bufs=2, space="PSUM") as pssc,
            tc.tile_pool(name="psav", bufs=1, space="PSUM") as psav,
            tc.tile_pool(name="pssum", bufs=1, space="PSUM") as pssum,
        ):
            qt = kq.tile([128, HEADS_PER_CORE * SEQ], F16, tag="qt", name="qt")
            kts = [kq.tile([128, CTX], F16, tag=f"kt{h}", name=f"kt{h}")
                   for h in range(HEADS_PER_CORE)]
            vps = [vv.tile([128, 8 * 1024], F16, tag=f"vp{p}", name=f"vp{p}")
                   for p in range(HEADS_PER_CORE // 2)]

            # ordered sync-queue loads: first-needed first
            # all input loads on the sync queue (hardware DGE; gpsimd and
            # scalar queues generate descriptors in software, ~20GB/s),
            # ordered so the first tiles of head 0 land first
            for (t, d, a, b) in [
                (kts[0], kt_d[0], 3840, 4096),   # tile 10 runs first
                (kts[0], kt_d[0], 0, 512),
                (qt, qt_d, 0, 512),
                (vps[0], vp_d[0], 7168, 8192),   # diag AV chunk
                (kts[0], kt_d[0], 512, 1536),
                (vps[0], vp_d[0], 0, 1024),
                (vps[0], vp_d[0], 1024, 2048),
                (kts[0], kt_d[0], 3584, 3840),   # tile 9 (pos 5)
                (vps[0], vp_d[0], 2048, 3072),
                (kts[0], kt_d[0], 1536, 2560),
                (vps[0], vp_d[0], 3072, 4096),
                (kts[0], kt_d[0], 2560, 3584),
                (vps[0], vp_d[0], 4096, 7168),
                (qt, qt_d, 512, 2048),
                (kts[1], kt_d[1], 0, 4096),
                (kts[2], kt_d[2], 0, 4096),
                (vps[1], vp_d[1], 0, 4096),
                (kts[3], kt_d[3], 0, 4096),
                (vps[1], vp_d[1], 4096, 8192),
            ]:
                nc.sync.dma_start(t[:, a:b], d[:, a:b])

            ones = consts.tile([128, 1], F16, tag="ones", name="ones")
            nc.vector.memset(ones[:], 1.0)

            def _vcol(h, tb):
                c, b = tb // 4, tb % 4
                return c * 1024 + b * 256 + (h % 2) * 128

            def emit_sum_block(jobs, sm, hh):
                assert len(jobs) == 13
                # full-width job first: start=True must zero the whole row
                jobs = ([j for j in jobs if j[1] == SEQ]
                        + [j for j in jobs if j[1] != SEQ])
                for i, (ap, n, col) in enumerate(jobs):
                    nc.tensor.matmul(sm[:, col:col + n], ones[:], ap,
                                     start=(i == 0), stop=(i == len(jobs) - 1))
                sm_sb = opool.tile([1, SEQ], F32, tag="smsb",
                                   name=f"smsb{hh}")
                nc.vector.tensor_copy(sm_sb[:], sm[:])
                nc.sync.dma_start(sums_d[hh], sm_sb[:])

            prev_sum = None
            for h in range(HEADS_PER_CORE):
                kt = kts[h]
                vp = vps[h // 2]
                qcol = h * SEQ
                av = psav.tile([128, SEQ], F32, tag="av", name=f"av{h}")
                sm = pssum.tile([1, SEQ], F32, tag="sm", name=f"sm{h}")

                sum_jobs = []   # deferred into the next head: PE must never
                                # wait mid-head on the fold chain
                u_list = []

                def do_sum(ap, n, col):
                    sum_jobs.append((ap, n, col))

                def emit_tail(ti, blocks, offs, widths, eap, first, last=False):
                    """AV + folds for tile ti (lagged one tile)."""
                    for j, tb in enumerate(blocks):
                        nc.tensor.matmul(
                            av[:, _soff(tb):SEQ],
                            vp[:, _vcol(h, tb):_vcol(h, tb) + 128],
                            eap(offs[j], offs[j] + widths[j]),
                            start=(first and j == 0),
                            stop=(last and j == len(blocks) - 1))
                    if ti <= 8:
                        feng = nc.vector
                        t1 = fpool.tile([128, SEQ], F16, tag="t1",
                                        name=f"t1_{h}_{ti}")
                        feng.tensor_add(t1[:], eap(0, 512), eap(512, 1024))
                        u = upool.tile([128, SEQ], F16, tag="u",
                                       name=f"u{h}_{ti}")
                        feng.tensor_add(u[:], t1[:], eap(1024, 1536))
                        u_list.append(u)
                    elif ti == 9:
                        u9 = upool.tile([128, SEQ], F16, tag="u9",
                                        name=f"u9_{h}")
                        nc.vector.tensor_add(u9[:], eap(0, 512),
                                             eap(512, 1024))
                        do_sum(u9[:], SEQ, 0)
                        do_sum(eap(1024, 1408), 384, 128)
                    else:
                        do_sum(eap(0, 256), 256, 256)
                        do_sum(eap(256, 384), 128, 384)

                pending = None
                off_tiles = OFF_TILES
                # tiny diag tile 10 first (fast sc turnaround at head
                # start), diag tile 9 mid-head so its serial chain hides
                for pos, ti in enumerate([10, 0, 1, 2, 3, 9, 4, 5, 6, 7, 8]):
                    blocks = TILES[ti]
                    if pos == 4 and prev_sum is not None:
                        # previous head's SUM matmuls: deferred here so PE
                        # never waits on the fold chain
                        emit_sum_block(*prev_sum)
                        prev_sum = None
                    widths = [_bw(tb) for tb in blocks]
                    offs = [int(sum(widths[:j])) for j in range(len(widths))]
                    tw = int(sum(widths))
                    sc = pssc.tile([128, 1536], F32, tag="sc",
                                   name=f"sc{h}_{ti}")
                    for j, tb in enumerate(blocks):
                        nc.tensor.matmul(
                            sc[:, offs[j]:offs[j] + widths[j]],
                            kt[:, tb * 128:(tb + 1) * 128],
                            qt[:, qcol + _soff(tb):qcol + SEQ],
                            start=True, stop=True)
                    if ti in off_tiles:
                        w16 = w16p.tile([128, 1536], I16, tag="w16",
                                        name=f"w16_{h}_{ti}")
                        nc.vector.tensor_scalar(
                            w16[:, 0:tw], sc[:, 0:tw], SCH_A16, SCH_B16,
                            MULT, ADD)
                        t_ = w16
                        eap = (lambda a, b, t_=t_: t_[:, a:b].bitcast(F16))
                    else:
                        e = epool.tile([128, 1536], F16, tag="e",
                                       name=f"e{h}_{ti}")
                        nc.scalar.activation(e[:, 0:tw], sc[:, 0:tw], EXP,
                                             scale=SCALE)
                        # causal mask on the first 128 queries of each diag
                        # block: zero e where query < key (gpsimd is idle)
                        for j, tb in enumerate(blocks):
                            if tb >= PREFIX_TB:
                                nc.gpsimd.affine_select(
                                    e[:, offs[j]:offs[j] + 128],
                                    e[:, offs[j]:offs[j] + 128],
                                    pattern=[[1, 128]],
                                    compare_op=mybir.AluOpType.is_ge,
                                    fill=0.0, base=0, channel_multiplier=-1)
                        eap = (lambda a, b, e=e: e[:, a:b])
                    if pending is not None:
                        emit_tail(*pending, last=False)
                    pending = (ti, blocks, offs, widths, eap, pos == 0)
                emit_tail(*pending, last=True)
                assert len(u_list) == 9
                for u in u_list:
                    sum_jobs.append((u[:], SEQ, 0))
                av_sb = opool.tile([128, SEQ], F32, tag="avsb",
                                   name=f"avsb{h}")
                nc.scalar.copy(av_sb[:], av[:])
                nc.sync.dma_start(outp_d[h], av_sb[:])
                prev_sum = (list(sum_jobs), sm, h)
            emit_sum_block(*prev_sum)

    nc.finalize()
    return nc


def _in_maps(query, key, value, kv_cache):
    bf = np.float16
    q_bf = query.astype(bf)                                        # [512, 4096]
    k_full = np.concatenate([kv_cache[0, :OFFSET], key], axis=0)   # [4096, 4096]
    v_full = np.concatenate([kv_cache[1, :OFFSET], value], axis=0)
    k_bf = k_full.astype(bf)
    v_bf = v_full.astype(bf)

    in_maps = []
    for c in range(N_CORES):
        cols = slice(c * CW, (c + 1) * CW)
        kt = np.ascontiguousarray(
            k_bf[:, cols].reshape(CTX, HEADS_PER_CORE, HEAD).transpose(1, 2, 0))
        qt = np.ascontiguousarray(
            q_bf[:, cols].reshape(SEQ, HEADS_PER_CORE, HEAD)
            .transpose(2, 1, 0).reshape(HEAD, HEADS_PER_CORE * SEQ))
        vpk = (v_bf[:, cols]
               .reshape(8, 4, 128, 2, 256)        # [c, b, p, pair, 256]
               .transpose(3, 2, 0, 1, 4)          # [pair, p, c, b, 256]
               .reshape(2, 128, 8 * 1024))
        in_maps.append({
            "qt": qt,
            "kt": kt,
            "vp": np.ascontiguousarray(vpk),
        })
    return in_maps


def _gather(res):
    """res.results[c] -> full [512, 4096] output (host does the division)."""
    outs = []
    for c in range(N_CORES):
        outp = res.results[c]["outp"]    # [4, 128, 512] fp32 (d, s)
        sums = res.results[c]["sums"]    # [4, 1, 512] fp32
        o = outp / sums
        outs.append(np.ascontiguousarray(
            o.transpose(2, 0, 1).reshape(SEQ, CW)))
    return np.concatenate(outs, axis=1)


def kernel(query, key, value, kv_cache, offset, seq_len):
    query = np.asarray(query, dtype=np.float32)
    key = np.asarray(key, dtype=np.float32)
    value = np.asarray(value, dtype=np.float32)
    kv_cache = np.asarray(kv_cache, dtype=np.float32)
    assert int(offset) == OFFSET and int(seq_len) == SEQ, (offset, seq_len)

    if "nc" not in _CACHE:
        _CACHE["nc"] = _build()
    nc = _CACHE["nc"]

    from concourse.bass_utils import run_bass_kernel_spmd

    res = run_bass_kernel_spmd(nc, _in_maps(query, key, value, kv_cache),
                               list(range(N_CORES)))
    return _gather(res)


# revision 66
# speedup vs baseline: 1.0144x; 1.0144x over previous
"""Sharded causal attention (decode-append) kernel for 8 NeuronCores, v3.

Head-parallel: core c owns heads 4c..4c+3. Per-core work: 4 heads x
[512 q] x [4096 ctx] x [128 d] QK/softmax/AV, fp16 operands, fp32 acc.

Design (vs the 118.5us v1 baseline):
- DMA: resident whole-tensor loads with 2-8KB rows (~250-370GB/s vs
  ~90GB/s for v1's 1KB-row chunk loads), one ordered sync queue whose
  first pieces are sized so head 0 can start ~1us after the ~3.4us
  framework preamble.
- Scores in [128,1536] PSUM tiles (3 t-blocks): 11 ScalarE exps per
  head instead of 18 (ACT is 0.83ns/elem + ~0.3us/instr and was the
  v1 co-bottleneck at 85us busy).
- Exp offload: the first 2 score tiles per head (t in [0,768)) are
  computed on the otherwise-overloaded-path-free DVE as an fp16-domain
  Schraudolph exp: int16(x*A16 + B16) bit-viewed as fp16. One
  tensor_scalar per tile, no convert pass (the int16 tile is fed to
  the AV matmul via .bitcast(f16) - verified legal on hw). Adds 8e-3
  rel err (gate 2e-2); C16 calibrated for truncating int conversion
  (hw matches numpy trunc - verified on v2).
- Softmax denominators: fold adds split between GpSimd (tiles 0-4,
  1.2us/add but otherwise idle) and DVE (rest, 0.55us/add), one
  ones-matmul per ~4 tiles into a [1,512] PSUM row.
- AV/folds lag the QK/exp of the next tile by one tile so PE never
  sits on an exp dependency (in-order engine queues).
- No on-chip epilogue: AV accumulators [d,s] are copied PSUM->SBUF on
  ScalarE, denominator rows on DVE, both DMAed raw; the host divides.
- PSUM: 2x sc(3 banks) + av(1) + sum(1) = 8 banks exactly.
"""

import sys

if "/opt/trn_rl_repo" not in sys.path:
    sys.path.insert(0, "/opt/trn_rl_repo")

import numpy as np

NUM_HEADS = 32
HEAD = 128
HIDDEN = NUM_HEADS * HEAD
MAX_SEQ = 4096
N_CORES = 8
HEADS_PER_CORE = NUM_HEADS // N_CORES          # 4
CW = HEADS_PER_CORE * HEAD                     # 512 columns per core
SEQ = 512
OFFSET = 3584
CTX = OFFSET + SEQ                             # 4096
TBLK = 128
NTB = CTX // TBLK                              # 32 t-blocks per head
PREFIX_TB = OFFSET // TBLK                     # 28 unmasked blocks
SCALE = float(1.0 / np.sqrt(np.float32(HEAD)))
MASK_NEG = -1.0e9

# Schraudolph fp16 exp offload: these 3-block tiles per head are exp'd
# on DVE instead of ScalarE. Late tiles chosen so the ScalarE act chain
# starts at tile 0 (it is the per-head critical path).
OFF_TILES = (4, 8)
SCH_C16 = 59.4
SCH_A16 = float(np.log2(np.e) * 1024.0 * SCALE)
SCH_B16 = float(15.0 * 1024.0 - SCH_C16)

TILES = ([[3 * i, 3 * i + 1, 3 * i + 2] for i in range(9)]
         + [[27, 28, 29], [30, 31]])

def _bw(tb):     # block width (query count)
    return SEQ - 128 * (tb - PREFIX_TB) if tb >= PREFIX_TB else SEQ

def _soff(tb):   # query offset
    return 128 * (tb - PREFIX_TB) if tb >= PREFIX_TB else 0

_CACHE: dict = {}


def _build():
    import concourse.bacc as bacc
    import concourse.tile as tile
    from concourse import mybir
    from concourse.vector_clock import ScopedClock

    def _lean_drain_and_barrier(self, tick_clock, wait_clock):
        nc = self.nc
        drain_inst = nc.sync.drain()
        wait_clock.add_sem_waits(
            drain_inst.ins, ScopedClock({None: tick_clock.global_clock}))
        nc.all_engine_barrier()
        popped = nc._tile_sem_poison_stack.pop()
        assert popped is self._sem_poison

        sems = list(self.sems.allocated().values())
        sem_nums = sorted(s.num if hasattr(s, "num") else s for s in sems)
        engines = [nc.gpsimd, nc.vector, nc.scalar, nc.tensor, nc.sync]
        ranges = []
        start = prev = None
        for n in sem_nums:
            if prev is None or n != prev + 1:
                if prev is not None:
                    ranges.append(range(start, prev + 1))
                start = n
            prev = n
        if prev is not None:
            ranges.append(range(start, prev + 1))
        for r in ranges:
            nc.gpsimd.dma_reset(r)
        chunks = []
        for r in ranges:
            vals = list(r)
            k = max(1, len(vals) // len(engines) + 1)
            for i in range(0, len(vals), k):
                seg = vals[i:i + k]
                chunks.append(range(seg[0], seg[-1] + 1))
        for i, r in enumerate(chunks):
            engines[i % len(engines)].sem_clear(r)
        nc._state.prepend_free_semaphores(sem_nums)
        for poison_set in nc._tile_sem_poison_stack:
            poison_set.update(sem_nums)

    tile.TileContext._drain_and_barrier = _lean_drain_and_barrier

    import concourse.bass as _bassmod
    _bassmod.is_customcomms_rdh_enabled = lambda: True

    F32 = mybir.dt.float32
    F16 = mybir.dt.float16
    I16 = mybir.dt.int16
    EXP = mybir.ActivationFunctionType.Exp
    ADD = mybir.AluOpType.add
    MULT = mybir.AluOpType.mult

    nc = bacc.Bacc()
    qt_d = nc.dram_tensor("qt", [128, HEADS_PER_CORE * SEQ], F16,
                          kind="ExternalInput")
    kt_d = nc.dram_tensor("kt", [HEADS_PER_CORE, 128, CTX], F16,
                          kind="ExternalInput")
    vp_d = nc.dram_tensor("vp", [HEADS_PER_CORE // 2, 128, 8 * 1024], F16,
                          kind="ExternalInput")
    outp_d = nc.dram_tensor("outp", [HEADS_PER_CORE, 128, SEQ], F32,
                            kind="ExternalOutput")
    sums_d = nc.dram_tensor("sums", [HEADS_PER_CORE, 1, SEQ], F32,
                            kind="ExternalOutput")

    with tile.TileContext(nc) as tc:
        with (
            tc.tile_pool(name="consts", bufs=1) as consts,
            tc.tile_pool(name="kq", bufs=1) as kq,
            tc.tile_pool(name="vv", bufs=1) as vv,
            tc.tile_pool(name="epool", bufs=8) as epool,
            tc.tile_pool(name="w16p", bufs=3) as w16p,
            tc.tile_pool(name="fpool", bufs=4) as fpool,
            tc.tile_pool(name="upool", bufs=12) as upool,
            tc.tile_pool(name="opool", bufs=2) as opool,
            tc.tile_pool(name="pssc", bufs=2, space="PSUM") as pssc,
            tc.tile_pool(name="psav", bufs=1, space="PSUM") as psav,
            tc.tile_pool(name="pssum", bufs=1, space="PSUM") as pssum,
        ):
            qt = kq.tile([128, HEADS_PER_CORE * SEQ], F16, tag="qt", name="qt")
            kts = [kq.tile([128, CTX], F16, tag=f"kt{h}", name=f"kt{h}")
                   for h in range(HEADS_PER_CORE)]
            vps = [vv.tile([128, 8 * 1024], F16, tag=f"vp{p}", name=f"vp{p}")
                   for p in range(HEADS_PER_CORE // 2)]

            # ordered sync-queue loads: first-needed first
            # all input loads on the sync queue (hardware DGE; gpsimd and
            # scalar queues generate descriptors in software, ~20GB/s),
            # ordered so the first tiles of head 0 land first
            for (t, d, a, b) in [
                (kts[0], kt_d[0], 3840, 4096),   # tile 10 runs first
                (kts[0], kt_d[0], 0, 512),
                (qt, qt_d, 0, 512),
                (vps[0], vp_d[0], 7168, 8192),   # diag AV chunk
                (kts[0], kt_d[0], 512, 1536),
                (vps[0], vp_d[0], 0, 1024),
                (vps[0], vp_d[0], 1024, 2048),
                (kts[0], kt_d[0], 3584, 3840),   # tile 9 (pos 5)
                (vps[0], vp_d[0], 2048, 3072),
                (kts[0], kt_d[0], 1536, 2560),
                (vps[0], vp_d[0], 3072, 4096),
                (kts[0], kt_d[0], 2560, 3584),
                (vps[0], vp_d[0], 4096, 7168),
                (qt, qt_d, 512, 2048),
                (kts[1], kt_d[1], 0, 4096),
                (kts[2], kt_d[2], 0, 4096),
                (vps[1], vp_d[1], 0, 4096),
                (kts[3], kt_d[3], 0, 4096),
                (vps[1], vp_d[1], 4096, 8192),
            ]:
                nc.sync.dma_start(t[:, a:b], d[:, a:b])

            ones = consts.tile([128, 1], F16, tag="ones", name="ones")
            nc.vector.memset(ones[:], 1.0)

            def _vcol(h, tb):
                c, b = tb // 4, tb % 4
                return c * 1024 + b * 256 + (h % 2) * 128

            def emit_sum_block(jobs, sm, hh):
                assert len(jobs) == 13
                # full-width job first: start=True must zero the whole row
                jobs = ([j for j in jobs if j[1] == SEQ]
                        + [j for j in jobs if j[1] != SEQ])
                for i, (ap, n, col) in enumerate(jobs):
                    nc.tensor.matmul(sm[:, col:col + n], ones[:], ap,
                                     start=(i == 0), stop=(i == len(jobs) - 1))
                sm_sb = opool.tile([1, SEQ], F32, tag="smsb",
                                   name=f"smsb{hh}")
                nc.vector.tensor_copy(sm_sb[:], sm[:])
                nc.sync.dma_start(sums_d[hh], sm_sb[:])

            prev_sum = None
            for h in range(HEADS_PER_CORE):
                kt = kts[h]
                vp = vps[h // 2]
                qcol = h * SEQ
                av = psav.tile([128, SEQ], F32, tag="av", name=f"av{h}")
                sm = pssum.tile([1, SEQ], F32, tag="sm", name=f"sm{h}")

                sum_jobs = []   # deferred into the next head: PE must never
                                # wait mid-head on the fold chain
                u_list = []

                def do_sum(ap, n, col):
                    sum_jobs.append((ap, n, col))

                def emit_tail(ti, blocks, offs, widths, eap, first, last=False):
                    """AV + folds for tile ti (lagged one tile)."""
                    for j, tb in enumerate(blocks):
                        nc.tensor.matmul(
                            av[:, _soff(tb):SEQ],
                            vp[:, _vcol(h, tb):_vcol(h, tb) + 128],
                            eap(offs[j], offs[j] + widths[j]),
                            start=(first and j == 0),
                            stop=(last and j == len(blocks) - 1))
                    if ti <= 8:
                        feng = nc.vector
                        t1 = fpool.tile([128, SEQ], F16, tag="t1",
                                        name=f"t1_{h}_{ti}")
                        feng.tensor_add(t1[:], eap(0, 512), eap(512, 1024))
                        u = upool.tile([128, SEQ], F16, tag="u",
                                       name=f"u{h}_{ti}")
                        feng.tensor_add(u[:], t1[:], eap(1024, 1536))
                        u_list.append(u)
                    elif ti == 9:
                        u9 = upool.tile([128, SEQ], F16, tag="u9",
                                        name=f"u9_{h}")
                        nc.vector.tensor_add(u9[:], eap(0, 512),
                                             eap(512, 1024))
                        do_sum(u9[:], SEQ, 0)
                        do_sum(eap(1024, 1408), 384, 128)
                    else:
                        do_sum(eap(0, 256), 256, 256)
                        do_sum(eap(256, 384), 128, 384)

                pending = None
                off_tiles = OFF_TILES
                # tiny diag tile 10 first (fast sc turnaround at head
                # start), diag tile 9 mid-head so its serial chain hides
                for pos, ti in enumerate([10, 0, 1, 2, 3, 9, 4, 5, 6, 7, 8]):
                    blocks = TILES[ti]
                    if pos == 4 and prev_sum is not None:
                        # previous head's SUM matmuls: deferred here so PE
                        # never waits on the fold chain
                        emit_sum_block(*prev_sum)
                        prev_sum = None
                    widths = [_bw(tb) for tb in blocks]
                    offs = [int(sum(widths[:j])) for j in range(len(widths))]
                    tw = int(sum(widths))
                    sc = pssc.tile([128, 1536], F32, tag="sc",
                                   name=f"sc{h}_{ti}")
                    for j, tb in enumerate(blocks):
                        nc.tensor.matmul(
                            sc[:, offs[j]:offs[j] + widths[j]],
                            kt[:, tb * 128:(tb + 1) * 128],
                            qt[:, qcol + _soff(tb):qcol + SEQ],
                            start=True, stop=True)
                    if ti in off_tiles:
                        w16 = w16p.tile([128, 1536], I16, tag="w16",
                                        name=f"w16_{h}_{ti}")
                        nc.vector.tensor_scalar(
                            w16[:, 0:tw], sc[:, 0:tw], SCH_A16, SCH_B16,
                            MULT, ADD)
                        t_ = w16
                        eap = (lambda a, b, t_=t_: t_[:, a:b].bitcast(F16))
                    else:
                        e = epool.tile([128, 1536], F16, tag="e",
                                       name=f"e{h}_{ti}")
                        nc.scalar.activation(e[:, 0:tw], sc[:, 0:tw], EXP,
                                             scale=SCALE)
                        # causal mask on the first 128 queries of each diag
                        # block: zero e where query < key (gpsimd is idle)
                        for j, tb in enumerate(blocks):
                            if tb >= PREFIX_TB:
                                nc.gpsimd.affine_select(
                                    e[:, offs[j]:offs[j] + 128],
                                    e[:, offs[j]:offs[j] + 128],
                                    pattern=[[1, 128]],
                                    compare_op=mybir.AluOpType.is_ge,
                                    fill=0.0, base=0, channel_multiplier=-1)
                        eap = (lambda a, b, e=e: e[:, a:b])
                    if pending is not None:
                        emit_tail(*pending, last=False)
                    pending = (ti, blocks, offs, widths, eap, pos == 0)
                emit_tail(*pending, last=True)
                assert len(u_list) == 9
                for u in u_list:
                    sum_jobs.append((u[:], SEQ, 0))
                av_sb = opool.tile([128, SEQ], F32, tag="avsb",
                                   name=f"avsb{h}")
                nc.scalar.copy(av_sb[:], av[:])
                nc.sync.dma_start(outp_d[h], av_sb[:])
                prev_sum = (list(sum_jobs), sm, h)
            emit_sum_block(*prev_sum)

    nc.finalize()
    return nc


def _in_maps(query, key, value, kv_cache):
    bf = np.float16
    q_bf = query.astype(bf)                                        # [512, 4096]
    k_full = np.concatenate([kv_cache[0, :OFFSET], key], axis=0)   # [4096, 4096]
    v_full = np.concatenate([kv_cache[1, :OFFSET], value], axis=0)
    k_bf = k_full.astype(bf)
    v_bf = v_full.astype(bf)

    in_maps = []
    for c in range(N_CORES):
        cols = slice(c * CW, (c + 1) * CW)
        kt = np.ascontiguousarray(
            k_bf[:, cols].reshape(CTX, HEADS_PER_CORE, HEAD).transpose(1, 2, 0))
        qt = np.ascontiguousarray(
            q_bf[:, cols].reshape(SEQ, HEADS_PER_CORE, HEAD)
            .transpose(2, 1, 0).reshape(HEAD, HEADS_PER_CORE * SEQ))
        vpk = (v_bf[:, cols]
               .reshape(8, 4, 128, 2, 256)        # [c, b, p, pair, 256]
               .transpose(3, 2, 0, 1, 4)          # [pair, p, c, b, 256]
               .reshape(2, 128, 8 * 1024))
        in_maps.append({
            "qt": qt,
            "kt": kt,
            "vp": np.ascontiguousarray(vpk),
        })
    return in_maps


def _gather(res):
    """res.results[c] -> full [512, 4096] output (host does the division)."""
    outs = []
    for c in range(N_CORES):
        outp = res.results[c]["outp"]    # [4, 128, 512] fp32 (d, s)
        sums = res.results[c]["sums"]    # [4, 1, 512] fp32
        o = outp / sums
        outs.append(np.ascontiguousarray(
            o.transpose(2, 0, 1).reshape(SEQ, CW)))
    return np.concatenate(outs, axis=1)


def kernel(query, key, value, kv_cache, offset, seq_len):
    query = np.asarray(query, dtype=np.float32)
    key = np.asarray(key, dtype=np.float32)
    value = np.asarray(value, dtype=np.float32)
    kv_cache = np.asarray(kv_cache, dtype=np.float32)
    assert int(offset) == OFFSET and int(seq_len) == SEQ, (offset, seq_len)

    if "nc" not in _CACHE:
        _CACHE["nc"] = _build()
    nc = _CACHE["nc"]

    from concourse.bass_utils import run_bass_kernel_spmd

    res = run_bass_kernel_spmd(nc, _in_maps(query, key, value, kv_cache),
                               list(range(N_CORES)))
    return _gather(res)


# revision 67
# speedup vs baseline: 1.0213x; 1.0068x over previous
"""Sharded causal attention (decode-append) kernel for 8 NeuronCores, v3.

Head-parallel: core c owns heads 4c..4c+3. Per-core work: 4 heads x
[512 q] x [4096 ctx] x [128 d] QK/softmax/AV, fp16 operands, fp32 acc.

Design (vs the 118.5us v1 baseline):
- DMA: resident whole-tensor loads with 2-8KB rows (~250-370GB/s vs
  ~90GB/s for v1's 1KB-row chunk loads), one ordered sync queue whose
  first pieces are sized so head 0 can start ~1us after the ~3.4us
  framework preamble.
- Scores in [128,1536] PSUM tiles (3 t-blocks): 11 ScalarE exps per
  head instead of 18 (ACT is 0.83ns/elem + ~0.3us/instr and was the
  v1 co-bottleneck at 85us busy).
- Exp offload: the first 2 score tiles per head (t in [0,768)) are
  computed on the otherwise-overloaded-path-free DVE as an fp16-domain
  Schraudolph exp: int16(x*A16 + B16) bit-viewed as fp16. One
  tensor_scalar per tile, no convert pass (the int16 tile is fed to
  the AV matmul via .bitcast(f16) - verified legal on hw). Adds 8e-3
  rel err (gate 2e-2); C16 calibrated for truncating int conversion
  (hw matches numpy trunc - verified on v2).
- Softmax denominators: fold adds split between GpSimd (tiles 0-4,
  1.2us/add but otherwise idle) and DVE (rest, 0.55us/add), one
  ones-matmul per ~4 tiles into a [1,512] PSUM row.
- AV/folds lag the QK/exp of the next tile by one tile so PE never
  sits on an exp dependency (in-order engine queues).
- No on-chip epilogue: AV accumulators [d,s] are copied PSUM->SBUF on
  ScalarE, denominator rows on DVE, both DMAed raw; the host divides.
- PSUM: 2x sc(3 banks) + av(1) + sum(1) = 8 banks exactly.
"""

import sys

if "/opt/trn_rl_repo" not in sys.path:
    sys.path.insert(0, "/opt/trn_rl_repo")

import numpy as np

NUM_HEADS = 32
HEAD = 128
HIDDEN = NUM_HEADS * HEAD
MAX_SEQ = 4096
N_CORES = 8
HEADS_PER_CORE = NUM_HEADS // N_CORES          # 4
CW = HEADS_PER_CORE * HEAD                     # 512 columns per core
SEQ = 512
OFFSET = 3584
CTX = OFFSET + SEQ                             # 4096
TBLK = 128
NTB = CTX // TBLK                              # 32 t-blocks per head
PREFIX_TB = OFFSET // TBLK                     # 28 unmasked blocks
SCALE = float(1.0 / np.sqrt(np.float32(HEAD)))
MASK_NEG = -1.0e9

# Schraudolph fp16 exp offload: these 3-block tiles per head are exp'd
# on DVE instead of ScalarE. Late tiles chosen so the ScalarE act chain
# starts at tile 0 (it is the per-head critical path).
OFF_TILES = (4, 8)
SCH_C16 = 59.4
SCH_A16 = float(np.log2(np.e) * 1024.0 * SCALE)
SCH_B16 = float(15.0 * 1024.0 - SCH_C16)

TILES = ([[3 * i, 3 * i + 1, 3 * i + 2] for i in range(9)]
         + [[27, 28, 29], [30, 31]])

def _bw(tb):     # block width (query count)
    return SEQ - 128 * (tb - PREFIX_TB) if tb >= PREFIX_TB else SEQ

def _soff(tb):   # query offset
    return 128 * (tb - PREFIX_TB) if tb >= PREFIX_TB else 0

_CACHE: dict = {}


def _build():
    import concourse.bacc as bacc
    import concourse.tile as tile
    from concourse import mybir
    from concourse.vector_clock import ScopedClock

    def _lean_drain_and_barrier(self, tick_clock, wait_clock):
        nc = self.nc
        drain_inst = nc.sync.drain()
        wait_clock.add_sem_waits(
            drain_inst.ins, ScopedClock({None: tick_clock.global_clock}))
        nc.all_engine_barrier()
        popped = nc._tile_sem_poison_stack.pop()
        assert popped is self._sem_poison

        sems = list(self.sems.allocated().values())
        sem_nums = sorted(s.num if hasattr(s, "num") else s for s in sems)
        engines = [nc.gpsimd, nc.vector, nc.scalar, nc.tensor, nc.sync]
        ranges = []
        start = prev = None
        for n in sem_nums:
            if prev is None or n != prev + 1:
                if prev is not None:
                    ranges.append(range(start, prev + 1))
                start = n
            prev = n
        if prev is not None:
            ranges.append(range(start, prev + 1))
        for r in ranges:
            nc.gpsimd.dma_reset(r)
        chunks = []
        for r in ranges:
            vals = list(r)
            k = max(1, len(vals) // len(engines) + 1)
            for i in range(0, len(vals), k):
                seg = vals[i:i + k]
                chunks.append(range(seg[0], seg[-1] + 1))
        for i, r in enumerate(chunks):
            engines[i % len(engines)].sem_clear(r)
        nc._state.prepend_free_semaphores(sem_nums)
        for poison_set in nc._tile_sem_poison_stack:
            poison_set.update(sem_nums)

    tile.TileContext._drain_and_barrier = _lean_drain_and_barrier

    import concourse.bass as _bassmod
    _bassmod.is_customcomms_rdh_enabled = lambda: True

    F32 = mybir.dt.float32
    F16 = mybir.dt.float16
    I16 = mybir.dt.int16
    EXP = mybir.ActivationFunctionType.Exp
    ADD = mybir.AluOpType.add
    MULT = mybir.AluOpType.mult

    nc = bacc.Bacc()
    qt_d = nc.dram_tensor("qt", [128, HEADS_PER_CORE * SEQ], F16,
                          kind="ExternalInput")
    kt_d = nc.dram_tensor("kt", [HEADS_PER_CORE, 128, CTX], F16,
                          kind="ExternalInput")
    vp_d = nc.dram_tensor("vp", [HEADS_PER_CORE // 2, 128, 8 * 1024], F16,
                          kind="ExternalInput")
    outp_d = nc.dram_tensor("outp", [HEADS_PER_CORE, 128, SEQ], F32,
                            kind="ExternalOutput")
    sums_d = nc.dram_tensor("sums", [HEADS_PER_CORE, 1, SEQ], F32,
                            kind="ExternalOutput")

    with tile.TileContext(nc) as tc:
        with (
            tc.tile_pool(name="consts", bufs=1) as consts,
            tc.tile_pool(name="kq", bufs=1) as kq,
            tc.tile_pool(name="vv", bufs=1) as vv,
            tc.tile_pool(name="epool", bufs=8) as epool,
            tc.tile_pool(name="w16p", bufs=3) as w16p,
            tc.tile_pool(name="fpool", bufs=4) as fpool,
            tc.tile_pool(name="upool", bufs=12) as upool,
            tc.tile_pool(name="opool", bufs=2) as opool,
            tc.tile_pool(name="pssc", bufs=2, space="PSUM") as pssc,
            tc.tile_pool(name="psav", bufs=1, space="PSUM") as psav,
            tc.tile_pool(name="pssum", bufs=1, space="PSUM") as pssum,
        ):
            qt = kq.tile([128, HEADS_PER_CORE * SEQ], F16, tag="qt", name="qt")
            kts = [kq.tile([128, CTX], F16, tag=f"kt{h}", name=f"kt{h}")
                   for h in range(HEADS_PER_CORE)]
            vps = [vv.tile([128, 8 * 1024], F16, tag=f"vp{p}", name=f"vp{p}")
                   for p in range(HEADS_PER_CORE // 2)]

            # ordered sync-queue loads: first-needed first
            # all input loads on the sync queue (hardware DGE; gpsimd and
            # scalar queues generate descriptors in software, ~20GB/s),
            # ordered so the first tiles of head 0 land first
            for (t, d, a, b) in [
                (kts[0], kt_d[0], 3840, 4096),   # tile 10 runs first
                (kts[0], kt_d[0], 0, 512),
                (qt, qt_d, 0, 512),
                (vps[0], vp_d[0], 7168, 8192),   # diag AV chunk
                (kts[0], kt_d[0], 512, 1536),
                (vps[0], vp_d[0], 0, 1024),
                (vps[0], vp_d[0], 1024, 2048),
                (kts[0], kt_d[0], 3584, 3840),   # tile 9 (pos 5)
                (vps[0], vp_d[0], 2048, 3072),
                (kts[0], kt_d[0], 1536, 2560),
                (vps[0], vp_d[0], 3072, 4096),
                (kts[0], kt_d[0], 2560, 3584),
                (vps[0], vp_d[0], 4096, 7168),
                (qt, qt_d, 512, 2048),
                (kts[1], kt_d[1], 0, 4096),
                (kts[2], kt_d[2], 0, 4096),
                (vps[1], vp_d[1], 0, 4096),
                (kts[3], kt_d[3], 0, 4096),
                (vps[1], vp_d[1], 4096, 8192),
            ]:
                nc.sync.dma_start(t[:, a:b], d[:, a:b])

            ones = consts.tile([128, 1], F16, tag="ones", name="ones")
            nc.vector.memset(ones[:], 1.0)

            def _vcol(h, tb):
                c, b = tb // 4, tb % 4
                return c * 1024 + b * 256 + (h % 2) * 128

            def emit_sum_block(jobs, sm, hh):
                assert len(jobs) in (9, 13)
                # full-width job first: start=True must zero the whole row
                jobs = ([j for j in jobs if j[1] == SEQ]
                        + [j for j in jobs if j[1] != SEQ])
                for i, (ap, n, col) in enumerate(jobs):
                    nc.tensor.matmul(sm[:, col:col + n], ones[:], ap,
                                     start=(i == 0), stop=(i == len(jobs) - 1))
                sm_sb = opool.tile([1, SEQ], F32, tag="smsb",
                                   name=f"smsb{hh}")
                nc.vector.tensor_copy(sm_sb[:], sm[:])
                nc.sync.dma_start(sums_d[hh], sm_sb[:])

            prev_sum = None
            for h in range(HEADS_PER_CORE):
                kt = kts[h]
                vp = vps[h // 2]
                qcol = h * SEQ
                av = psav.tile([128, SEQ], F32, tag="av", name=f"av{h}")
                sm = pssum.tile([1, SEQ], F32, tag="sm", name=f"sm{h}")

                sum_jobs = []   # deferred into the next head: PE must never
                                # wait mid-head on the fold chain
                u_list = []

                def do_sum(ap, n, col):
                    sum_jobs.append((ap, n, col))

                def emit_tail(ti, blocks, offs, widths, eap, first, last=False):
                    """AV + folds for tile ti (lagged one tile)."""
                    for j, tb in enumerate(blocks):
                        nc.tensor.matmul(
                            av[:, _soff(tb):SEQ],
                            vp[:, _vcol(h, tb):_vcol(h, tb) + 128],
                            eap(offs[j], offs[j] + widths[j]),
                            start=(first and j == 0),
                            stop=(last and j == len(blocks) - 1))
                    if ti <= 8:
                        feng = nc.vector
                        t1 = fpool.tile([128, SEQ], F16, tag="t1",
                                        name=f"t1_{h}_{ti}")
                        feng.tensor_add(t1[:], eap(0, 512), eap(512, 1024))
                        u = upool.tile([128, SEQ], F16, tag="u",
                                       name=f"u{h}_{ti}")
                        feng.tensor_add(u[:], t1[:], eap(1024, 1536))
                        u_list.append(u)
                    elif ti == 9:
                        u9 = upool.tile([128, SEQ], F16, tag="u9",
                                        name=f"u9_{h}")
                        nc.vector.tensor_add(u9[:], eap(0, 512),
                                             eap(512, 1024))
                        do_sum(u9[:], SEQ, 0)
                        do_sum(eap(1024, 1408), 384, 128)
                    else:
                        do_sum(eap(0, 256), 256, 256)
                        do_sum(eap(256, 384), 128, 384)

                pending = None
                off_tiles = OFF_TILES
                # tiny diag tile 10 first (fast sc turnaround at head
                # start), diag tile 9 mid-head so its serial chain hides
                for pos, ti in enumerate([10, 0, 1, 2, 3, 9, 4, 5, 6, 7, 8]):
                    blocks = TILES[ti]
                    if pos == 2 and prev_sum is not None:
                        # merge the previous head's u tiles pairwise on DVE
                        # (slack here); its SUM block runs at pos 4
                        pj, pu, psm, ph = prev_sum
                        for k in range(4):
                            w = upool.tile([128, SEQ], F16, tag="w",
                                           name=f"w{ph}_{k}")
                            nc.vector.tensor_add(w[:], pu[2 * k][:],
                                                 pu[2 * k + 1][:])
                            pj.append((w[:], SEQ, 0))
                        pj.append((pu[8][:], SEQ, 0))
                        prev_sum = (pj, psm, ph)
                    if pos == 4 and prev_sum is not None:
                        # previous head's SUM matmuls: deferred here so PE
                        # never waits on the fold chain
                        emit_sum_block(*prev_sum)
                        prev_sum = None
                    widths = [_bw(tb) for tb in blocks]
                    offs = [int(sum(widths[:j])) for j in range(len(widths))]
                    tw = int(sum(widths))
                    sc = pssc.tile([128, 1536], F32, tag="sc",
                                   name=f"sc{h}_{ti}")
                    for j, tb in enumerate(blocks):
                        nc.tensor.matmul(
                            sc[:, offs[j]:offs[j] + widths[j]],
                            kt[:, tb * 128:(tb + 1) * 128],
                            qt[:, qcol + _soff(tb):qcol + SEQ],
                            start=True, stop=True)
                    if ti in off_tiles:
                        w16 = w16p.tile([128, 1536], I16, tag="w16",
                                        name=f"w16_{h}_{ti}")
                        nc.vector.tensor_scalar(
                            w16[:, 0:tw], sc[:, 0:tw], SCH_A16, SCH_B16,
                            MULT, ADD)
                        t_ = w16
                        eap = (lambda a, b, t_=t_: t_[:, a:b].bitcast(F16))
                    else:
                        e = epool.tile([128, 1536], F16, tag="e",
                                       name=f"e{h}_{ti}")
                        nc.scalar.activation(e[:, 0:tw], sc[:, 0:tw], EXP,
                                             scale=SCALE)
                        # causal mask on the first 128 queries of each diag
                        # block: zero e where query < key (gpsimd is idle)
                        for j, tb in enumerate(blocks):
                            if tb >= PREFIX_TB:
                                nc.gpsimd.affine_select(
                                    e[:, offs[j]:offs[j] + 128],
                                    e[:, offs[j]:offs[j] + 128],
                                    pattern=[[1, 128]],
                                    compare_op=mybir.AluOpType.is_ge,
                                    fill=0.0, base=0, channel_multiplier=-1)
                        eap = (lambda a, b, e=e: e[:, a:b])
                    if pending is not None:
                        emit_tail(*pending, last=False)
                    pending = (ti, blocks, offs, widths, eap, pos == 0)
                emit_tail(*pending, last=True)
                assert len(u_list) == 9
                av_sb = opool.tile([128, SEQ], F32, tag="avsb",
                                   name=f"avsb{h}")
                nc.scalar.copy(av_sb[:], av[:])
                nc.sync.dma_start(outp_d[h], av_sb[:])
                if h < HEADS_PER_CORE - 1:
                    prev_sum = (list(sum_jobs), list(u_list), sm, h)
                else:
                    # last head: direct sums at the tail (deps all ready)
                    for u in u_list:
                        sum_jobs.append((u[:], SEQ, 0))
                    emit_sum_block(list(sum_jobs), sm, h)

    nc.finalize()
    return nc


def _in_maps(query, key, value, kv_cache):
    bf = np.float16
    q_bf = query.astype(bf)                                        # [512, 4096]
    k_full = np.concatenate([kv_cache[0, :OFFSET], key], axis=0)   # [4096, 4096]
    v_full = np.concatenate([kv_cache[1, :OFFSET], value], axis=0)
    k_bf = k_full.astype(bf)
    v_bf = v_full.astype(bf)

    in_maps = []
    for c in range(N_CORES):
        cols = slice(c * CW, (c + 1) * CW)
        kt = np.ascontiguousarray(
            k_bf[:, cols].reshape(CTX, HEADS_PER_CORE, HEAD).transpose(1, 2, 0))
        qt = np.ascontiguousarray(
            q_bf[:, cols].reshape(SEQ, HEADS_PER_CORE, HEAD)
            .transpose(2, 1, 0).reshape(HEAD, HEADS_PER_CORE * SEQ))
        vpk = (v_bf[:, cols]
               .reshape(8, 4, 128, 2, 256)        # [c, b, p, pair, 256]
               .transpose(3, 2, 0, 1, 4)          # [pair, p, c, b, 256]
               .reshape(2, 128, 8 * 1024))
        in_maps.append({
            "qt": qt,
            "kt": kt,
            "vp": np.ascontiguousarray(vpk),
        })
    return in_maps


def _gather(res):
    """res.results[c] -> full [512, 4096] output (host does the division)."""
    outs = []
    for c in range(N_CORES):
        outp = res.results[c]["outp"]    # [4, 128, 512] fp32 (d, s)
        sums = res.results[c]["sums"]    # [4, 1, 512] fp32
        o = outp / sums
        outs.append(np.ascontiguousarray(
            o.transpose(2, 0, 1).reshape(SEQ, CW)))
    return np.concatenate(outs, axis=1)


def kernel(query, key, value, kv_cache, offset, seq_len):
    query = np.asarray(query, dtype=np.float32)
    key = np.asarray(key, dtype=np.float32)
    value = np.asarray(value, dtype=np.float32)
    kv_cache = np.asarray(kv_cache, dtype=np.float32)
    assert int(offset) == OFFSET and int(seq_len) == SEQ, (offset, seq_len)

    if "nc" not in _CACHE:
        _CACHE["nc"] = _build()
    nc = _CACHE["nc"]

    from concourse.bass_utils import run_bass_kernel_spmd

    res = run_bass_kernel_spmd(nc, _in_maps(query, key, value, kv_cache),
                               list(range(N_CORES)))
    return _gather(res)
